# revision 82
# baseline (speedup 1.0000x reference)
"""Multi-head attention (B=2, S=2048, H=1024, 16 heads) on 8 TRN2 NeuronCores.

Sharding: data-parallel over batch (2) x tensor-parallel over heads (16 -> 4
groups of 4 heads).  Core c = b*4 + g handles batch b, heads [4g, 4g+4).

Per-core math (fp16 storage / fp32 accumulate), with x = q|k|v of its batch:
  QP_T[d, s] = (Wq_g x^T + bq_g)   stored transposed, d on partitions
  KP_T[d, s] = (Wk_g x^T + bk_g)
  VP[s, d]   = x Wv_g^T            natural layout  (bv folded on host)
  per head h:  S_T[j, i] = KP_T_h^T-contracted scores (d contracts)
               A = exp(S_T / 8)                       (softmax w/o max-sub)
               O_T[d, i] = VP_h^T A   and  L[i] = ones^T A  (via ones col)
               O_norm = O_T * (1/L)   (1/L = exp(-ln L) on ACT, one row;
                                       partition-broadcast via DRAM DMA)
  out_T[o, i] = Wo_g^T-contracted projection of O_norm   -> [1024, 2048] f32
Host: out[b] = sum_g out_T(b,g)^T + (Wo @ bv + bo).

Schedule notes (from perfetto iteration):
- The scalar-engine exp stream (~16.8M exps/core ~= 147us) is the hard
  bottleneck; scores+exp run at high priority, everything else fills.
- Norm chains run at medium priority and drift a few slots into the next
  call; oa/ob PSUM is released early via an SBUF copy (oacp).
- Inputs are host-pre-rearranged so every DMA row is contiguous (128
  descriptors/transfer), ordered by consumption deadline; dummy exp warms
  the ACT table and dummy matmuls warm the PE clock during the DMA head.
"""

import json

import numpy as np

S = 2048
H = 1024
DL = 256          # local projection dim = 4 heads * 64
P = 128
HD = 64
NK = H // P       # 8 k-tiles over hidden dim
NI = 4            # i blocks of 512 queries
NJ = S // P       # 16 j tiles of 128 keys
NB = 512          # free-dim block

_nc_cache = {}


# --------------------------------------------------------------------------
# BIR fix: this container's walrus supports only ONE sync wait (and update)
# per TPB instruction; Tile attaches several.  Split extras onto single-wait
# EventSemaphore instructions at the serialization boundary.
# --------------------------------------------------------------------------
_wsplit_counter = [0]


def _mk_evsem(engine, debug, wait=None, update=None):
    _wsplit_counter[0] += 1
    return {
        "debug": debug,
        "engine": engine,
        "ins": [],
        "outs": [],
        "name": f"wsplit-{_wsplit_counter[0]}",
        "opcode": "EventSemaphore",
        "sync_info": {
            "on_wait": [wait] if wait else [],
            "on_update": [update] if update else [],
        },
    }


def _split_bir_waits(bir):
    for f in bir.get("functions", []):
        for blk in f.get("blocks", []):
            out = []
            for inst in blk.get("instructions", []):
                si = inst.get("sync_info")
                waits = list(si.get("on_wait") or []) if si else []
                updates = list(si.get("on_update") or []) if si else []
                eng = inst.get("engine")
                dbg = inst.get("debug", 0)
                if len(waits) > 1:
                    for w in waits[:-1]:
                        out.append(_mk_evsem(eng, dbg, wait=w))
                    si["on_wait"] = [waits[-1]]
                out.append(inst)
                if len(updates) > 1:
                    si["on_update"] = [updates[0]]
                    for u in updates[1:]:
                        out.append(_mk_evsem(eng, dbg, update=u))
            blk["instructions"] = out
    return bir


def _install_bir_fix():
    import concourse.bass as bass

    if getattr(bass.Bass, "_wsplit_installed", False):
        return
    orig = bass.Bass.to_json_bytes

    def to_json_bytes(self, *a, **k):
        bir = json.loads(orig(self, *a, **k))
        return json.dumps(_split_bir_waits(bir)).encode()

    bass.Bass.to_json_bytes = to_json_bytes
    bass.Bass._wsplit_installed = True


# --------------------------------------------------------------------------
# Kernel builder
# --------------------------------------------------------------------------

def _build_nc():
    import concourse.bass as bass
    import concourse.mybir as mybir
    import concourse.tile as tile

    f16 = mybir.dt.float16
    f32 = mybir.dt.float32
    f32r = mybir.dt.float32r
    Exp = mybir.ActivationFunctionType.Exp
    Ln = mybir.ActivationFunctionType.Ln
    VW = 4 * (HD + 1) + 63  # vp row width: 4 heads x 65 + pad to let lhsT span 128

    nc = bass.Bass("TRN2")

    # inputs arrive pre-rearranged on the host so every DMA row is contiguous
    # (128 descriptors per transfer instead of 1024 -> ~5x cheaper issue)
    xqd = [nc.dram_tensor(f"xq{n}", [P, NK * NB], f16, kind="ExternalInput")
           for n in range(NI)]
    xkd = [nc.dram_tensor(f"xk{n}", [P, NK * NB], f16, kind="ExternalInput")
           for n in range(NI)]
    xvd = [nc.dram_tensor(f"xv{n}", [P, NK * NB], f16, kind="ExternalInput")
           for n in range(NI)]
    wqT = nc.dram_tensor("wqT", [P, NK * DL], f16, kind="ExternalInput")
    wkT = nc.dram_tensor("wkT", [P, NK * DL], f16, kind="ExternalInput")
    wvT = nc.dram_tensor("wvT", [P, NK * DL], f16, kind="ExternalInput")
    woT = nc.dram_tensor("woT", [P, 2 * H], f16, kind="ExternalInput")
    bias = nc.dram_tensor("bias", [P, 4], f32, kind="ExternalInput")  # bq0 bq1 bk0 bk1
    out = nc.dram_tensor("out", [H, S], f16, kind="ExternalOutput")
    # per-call 1/l staging row for the partition-broadcast DMA
    rsc = [nc.dram_tensor(f"rsc{i}", [1, 2 * NB], f16, kind="Internal")
           for i in range(8)]

    with tile.TileContext(nc) as tc:
        with (
            tc.tile_pool(name="persist", bufs=1) as persist,
            tc.tile_pool(name="xpool", bufs=1) as xpool,
            tc.tile_pool(name="exppool", bufs=8) as exppool,
            tc.tile_pool(name="lrpool", bufs=1) as lrpool,
            tc.tile_pool(name="oacpool", bufs=2) as oacpool,
            tc.tile_pool(name="evpool", bufs=4) as evpool,
            tc.tile_pool(name="warmpool", bufs=1) as warmpool,
            tc.tile_pool(name="scps", bufs=2, space="PSUM") as scps,
            tc.tile_pool(name="oaob", bufs=1, space="PSUM") as oaob,
            tc.tile_pool(name="trans", bufs=2, space="PSUM") as trans,
        ):
            # ---- persistent tiles ----
            wq_sb = persist.tile([P, NK, DL], f16, name="wq_sb")
            wk_sb = persist.tile([P, NK, DL], f16, name="wk_sb")
            wv_sb = persist.tile([P, NK, DL], f16, name="wv_sb")
            wo_sb = persist.tile([P, 2, H], f16, name="wo_sb")
            wrm16 = persist.tile([P, NB], f16, name="wrm16")
            bias_sb = persist.tile([P, 4], f32, name="bias_sb")
            ones_f32 = persist.tile([P, P], f32, name="ones_f32")
            onesrow = persist.tile([P, P], f32r, name="onesrow")
            qpt = persist.tile([P, 2, S], f16, name="qpt")
            kpt = persist.tile([P, 2, S], f16, name="kpt")
            vp = persist.tile([P, NJ, VW], f16, name="vp")
            onorm = persist.tile([P, 2, S], f16, name="onorm")

            # ---- no-dependency setup: constants + ACT exp-table prefetch ----
            nc.gpsimd.memset(ones_f32[:], 1.0)
            nc.vector.tensor_copy(onesrow[:], ones_f32[:])
            for h in range(4):
                nc.gpsimd.memset(vp[:, :, h * (HD + 1) + HD:h * (HD + 1) + HD + 1], 1.0)
            nc.gpsimd.memset(vp[:, :, 4 * (HD + 1):], 0.0)
            # dummy exp warms the ACT table RAM (~2.7us) off the critical path
            warm = warmpool.tile([P, NB], f16, name="warm_t")
            nc.scalar.activation(warm[0:1, 0:16], ones_f32[0:1, 0:16], Exp)
            # dummy matmuls warm the PE clock (HAM K=8/8) during the DMA wait
            nc.gpsimd.memset(wrm16[:], 0.0)
            wps = trans.tile([P, NB], f32, name="tr_t")
            for i in range(12):
                nc.tensor.matmul(wps[:], wrm16[:, 0:P], wrm16[:],
                                 start=(i == 0), stop=(i == 11))
            nc.vector.tensor_copy(warm[0:1, 256:], wps[0:1, 256:])

            def warm2():
                # keep the PE busy between K proj and the xq0 arrival
                w2 = trans.tile([P, NB], f32, name="tr_t")
                for i in range(10):
                    nc.tensor.matmul(w2[:], wrm16[:, 0:P], wrm16[:],
                                     start=(i == 0), stop=(i == 9))
                nc.vector.tensor_copy(warm[0:1, 0:256], w2[0:1, 0:256])

            # ---- DMA emission, ordered by consumption deadline ----
            xk_t = {}
            xq_t = {}
            xv_t = {}

            def load_quarter(dst, src, tag, n, eng=None):
                t = xpool.tile([P, NK, NB], f16, name=f"x{tag}{n}")
                (eng or nc.sync).dma_start(
                    t[:].rearrange("p k s -> p (k s)"), src[n][:])
                dst[n] = t

            nc.sync.dma_start(bias_sb[:], bias[:])
            nc.sync.dma_start(wk_sb[:].rearrange("p k d -> p (k d)"), wkT[:])
            load_quarter(xk_t, xkd, "k", 0)
            nc.sync.dma_start(wq_sb[:].rearrange("p k d -> p (k d)"), wqT[:])
            load_quarter(xq_t, xqd, "q", 0)
            load_quarter(xk_t, xkd, "k", 1)
            nc.sync.dma_start(wv_sb[:].rearrange("p k d -> p (k d)"), wvT[:])
            load_quarter(xv_t, xvd, "v", 0)
            load_quarter(xk_t, xkd, "k", 2)
            load_quarter(xv_t, xvd, "v", 1)
            load_quarter(xk_t, xkd, "k", 3)
            load_quarter(xv_t, xvd, "v", 2)
            load_quarter(xq_t, xqd, "q", 1)
            load_quarter(xv_t, xvd, "v", 3)
            load_quarter(xq_t, xqd, "q", 2)
            load_quarter(xq_t, xqd, "q", 3)
            nc.sync.dma_start(wo_sb[:].rearrange("p k d -> p (k d)"), woT[:])

            # ---- V projection for one s-tile ----
            def v_block(s):
                n, c = divmod(s, 4)
                ps = trans.tile([P, NB], f32, name="tr_t")
                for k in range(NK):
                    nc.tensor.matmul(
                        ps[:, :DL],
                        xv_t[n][:, k, c * P:(c + 1) * P],
                        wv_sb[:, k, :],
                        start=(k == 0),
                        stop=(k == NK - 1),
                    )
                dst = vp[:, s, 0:4 * (HD + 1)].rearrange(
                    "p (h d) -> p h d", h=4)[:, :, 0:HD]
                nc.vector.tensor_copy(dst, ps[:, :DL].rearrange("p (h d) -> p h d", h=4))

            # ---- K/Q projection, one (m, n) block ----
            def proj_qk_n(wsb, xt, dst, bcol, m, n):
                ps = trans.tile([P, NB], f32, name="tr_t")
                for k in range(NK):
                    nc.tensor.matmul(
                        ps[:],
                        wsb[:, k, m * P:(m + 1) * P],
                        xt[n][:, k, :],
                        start=(k == 0),
                        stop=(k == NK - 1),
                    )
                nc.vector.tensor_scalar_add(
                    dst[:, m, n * NB:(n + 1) * NB], ps[:],
                    bias_sb[:, bcol + m:bcol + m + 1],
                )

            # ---- output projection, one mo chunk of one n block ----
            def wo_chunk(n, mo, ceng=None):
                ps = trans.tile([P, NB], f32, name="tr_t")
                for k2 in range(2):
                    nc.tensor.matmul(
                        ps[:],
                        wo_sb[:, k2, mo * P:(mo + 1) * P],
                        onorm[:, k2, n * NB:(n + 1) * NB],
                        start=(k2 == 0),
                        stop=(k2 == 1),
                    )
                ot = evpool.tile([P, NB], f16, name="ot_t")
                if ceng is None:
                    nc.vector.tensor_copy(ot[:], ps[:])
                else:
                    nc.scalar.copy(ot[:], ps[:])
                nc.sync.dma_start(out[mo * P:(mo + 1) * P, n * NB:(n + 1) * NB], ot[:])

            HP = 3000   # priority offset: scores/exp chain preempts fill work
            HP2 = 2930  # norm chain: a few slots into the next call's stream

            def attention_ib(p, ib, fills=None, norm_hp=None, last=False):
                isl = slice(ib * NB, (ib + 1) * NB)
                oa = oaob.tile([P, NB], f32, name="oa_t")
                ob = oaob.tile([P, NB], f32, name="ob_t")
                es = []

                def emit_av(jb):
                    e = es[jb]
                    offa = 2 * p * (HD + 1)
                    offb = (2 * p + 1) * (HD + 1)
                    nc.tensor.matmul(
                        oa[:, :], vp[:, jb, offa:offa + P], e[:, 0:NB],
                        start=(jb == 0), stop=(jb == NJ - 1),
                    )
                    nc.tensor.matmul(
                        ob[:, :], vp[:, jb, offb:offb + P], e[:, NB:2 * NB],
                        start=(jb == 0), stop=(jb == NJ - 1),
                    )

                # one-stage software pipeline: QK(jb) ahead of AV(jb-1)
                for jb in range(NJ):
                    jsl = slice(jb * P, (jb + 1) * P)
                    sc = scps.tile([P, 2 * NB], f32, name="sc_t")
                    with tc.high_priority(offset=HP):
                        nc.tensor.matmul(
                            sc[:, 0:NB], kpt[0:HD, p, jsl], qpt[0:HD, p, isl],
                            start=True, stop=True,
                        )
                        nc.tensor.matmul(
                            sc[:, NB:2 * NB], kpt[HD:P, p, jsl], qpt[HD:P, p, isl],
                            start=True, stop=True,
                        )
                        e = exppool.tile([P, 2 * NB], f16, name="e_t")
                        nc.scalar.activation(e[:], sc[:], Exp, scale=0.125)
                    es.append(e)
                    if fills is not None and jb in fills:
                        for th in fills[jb]:
                            th()
                    if jb >= 1:
                        emit_av(jb - 1)
                emit_av(NJ - 1)

                if last:
                    # tail call: PSUM is free and latency is king — broadcast
                    # l with K=1 matmuls, 1/l on the broadcast tile, multiply.
                    with tc.high_priority(offset=HP):
                        lsb = lrpool.tile([P, 2 * NB], f32r, name="lsb_t")
                        with nc.allow_low_precision(reason="denom via f32r"):
                            nc.vector.tensor_copy(lsb[HD:HD + 1, 0:NB],
                                                  oa[HD:HD + 1, :])
                            nc.vector.tensor_copy(lsb[HD:HD + 1, NB:2 * NB],
                                                  ob[HD:HD + 1, :])
                        rb = scps.tile([P, 2 * NB], f32, name="sc_t")
                        nc.tensor.matmul(
                            rb[:, 0:NB], onesrow[HD:HD + 1, :],
                            lsb[HD:HD + 1, 0:NB], start=True, stop=True)
                        nc.tensor.matmul(
                            rb[:, NB:2 * NB], onesrow[HD:HD + 1, :],
                            lsb[HD:HD + 1, NB:2 * NB], start=True, stop=True)
                        lnb = lrpool.tile([P, 2 * NB], f32, name="lnb_t")
                        nc.scalar.activation(lnb[:], rb[:], Ln)
                        rbs = lrpool.tile([P, 2 * NB], f32, name="rbs_t")
                        nc.scalar.activation(rbs[:], lnb[:], Exp, scale=-1.0)
                        tmpb = evpool.tile([P, NB], f16, name="tmpb_t")
                        nc.vector.tensor_mul(tmpb[0:HD, :], ob[0:HD, :],
                                             rbs[0:HD, NB:2 * NB])
                        nc.sync.dma_start(onorm[HD:P, p, isl], tmpb[0:HD, :])
                        nc.vector.tensor_mul(onorm[0:HD, p, isl], oa[0:HD, :],
                                             rbs[0:HD, 0:NB])
                    return

                # normalization: l sits at row HD of oa/ob.  Copy O|l out of
                # PSUM immediately (releases oa/ob for the next call), then
                # 1/l = exp(-ln l) on the single l row (ACT, natural_log_exp
                # set), broadcast r across partitions via a DRAM-staged DMA,
                # multiply on the DVE.  Medium priority: next call's exp
                # stream preempts, the chain fills ACT/PE bubbles.
                with tc.high_priority(offset=HP):
                    oacp = oacpool.tile([P, 2 * NB], f32, name="oacp_t")
                    nc.vector.tensor_copy(oacp[0:HD + 1, 0:NB], oa[0:HD + 1, :])
                    nc.vector.tensor_copy(oacp[0:HD + 1, NB:2 * NB], ob[0:HD + 1, :])
                with tc.high_priority(offset=HP2 if norm_hp is None else norm_hp):
                    lrow = lrpool.tile([P, 2 * NB], f32, name="lrow_t")
                    nc.scalar.activation(lrow[HD:HD + 1, :], oacp[HD:HD + 1, :], Ln)
                    rrow = lrpool.tile([P, 2 * NB], f32, name="rrow_t")
                    nc.scalar.activation(rrow[HD:HD + 1, :], lrow[HD:HD + 1, :], Exp,
                                         scale=-1.0)
                    # partition-broadcast 1/l: SWDGE cast-DMA to a DRAM row,
                    # then a zero-stride read fans it across 128 partitions
                    # (SBUF sources cannot have partition step 0; DRAM can)
                    rdram = rsc[4 * p + ib]
                    nc.gpsimd.dma_start(rdram[:], rrow[HD:HD + 1, :])
                    rbsb = lrpool.tile([P, 2 * NB], f16, name="rbsb_t")
                    nc.gpsimd.dma_start(
                        rbsb[:], rdram[:].to_broadcast([P, 2 * NB]))
                    tmpb = evpool.tile([P, NB], f16, name="tmpb_t")
                    nc.vector.tensor_mul(
                        tmpb[0:HD, :], oacp[0:HD, NB:2 * NB], rbsb[0:HD, NB:2 * NB])
                    nc.sync.dma_start(onorm[HD:P, p, isl], tmpb[0:HD, :])
                    nc.vector.tensor_mul(
                        onorm[0:HD, p, isl], oacp[0:HD, 0:NB], rbsb[0:HD, 0:NB])

            # ---- schedule ----
            def P_(wsb, xd, dst, bcol, m, n):
                return lambda: proj_qk_n(wsb, xd, dst, bcol, m, n)

            K0 = lambda n: P_(wk_sb, xk_t, kpt, 2, 0, n)
            Q0 = lambda n: P_(wq_sb, xq_t, qpt, 0, 0, n)
            K1 = lambda n: P_(wk_sb, xk_t, kpt, 2, 1, n)
            Q1 = lambda n: P_(wq_sb, xq_t, qpt, 0, 1, n)
            V_ = lambda s: (lambda: v_block(s))
            WO = lambda n, mo: (lambda: wo_chunk(n, mo))

            def wo_fills(n):
                return {2 * mo + 1: [WO(n, mo)] for mo in range(8)}

            # prologue: only what gates the first exp; V rides as call-0 fills
            K0(0)()
            warm2()
            Q0(0)()

            attention_ib(0, 0, fills={
                0: [K0(1)],
                1: [V_(0)], 2: [V_(1)], 3: [V_(2)],
                4: [K0(2), V_(3)],
                5: [V_(4), V_(5)],
                6: [V_(6)], 7: [V_(7)],
                8: [K0(3), V_(8)],
                9: [V_(9)], 10: [V_(10)], 11: [V_(11)],
                12: [Q0(1), V_(12)],
                13: [V_(13)], 14: [V_(14)], 15: [V_(15)],
            })
            attention_ib(0, 1, fills={2: [Q0(2)], 8: [K1(0)], 12: [K1(1)]})
            attention_ib(0, 2, fills={2: [Q0(3)], 8: [K1(2)], 12: [K1(3)]})
            attention_ib(0, 3, fills={2: [Q1(0)], 8: [Q1(1)], 12: [Q1(2)]})
            attention_ib(1, 0, fills={2: [Q1(3)]})
            attention_ib(1, 1, fills=wo_fills(0))
            attention_ib(1, 2, fills=wo_fills(1))
            attention_ib(1, 3, fills={jb + 7: [WO(2, jb)] for jb in range(8)},
                         last=True)
            # tail PE warmers from the now-free scps pool: keep HAM at 8/8
            # through the last norm chain so wo(3) runs at full clock
            wps3 = scps.tile([P, 2 * NB], f32, name="sc_t")
            for i in range(14):
                nc.tensor.matmul(wps3[:, 0:NB], wrm16[:, 0:P], wrm16[:],
                                 start=(i == 0), stop=(i == 13))
            nc.vector.tensor_copy(warm[0:1, 0:128], wps3[0:1, 0:128])
            for mo in range(8):
                wo_chunk(3, mo)

    return nc


def _get_nc():
    if "nc" not in _nc_cache:
        _install_bir_fix()
        _nc_cache["nc"] = _build_nc()
    return _nc_cache["nc"]


# --------------------------------------------------------------------------
# Host wrapper
# --------------------------------------------------------------------------
def run(inputs, trace=False):
    from concourse.bass_utils import run_bass_kernel_spmd

    q = np.asarray(inputs["q"], np.float32)
    k = np.asarray(inputs["k"], np.float32)
    v = np.asarray(inputs["v"], np.float32)
    Wq = np.asarray(inputs["Wq"], np.float32)
    bq = np.asarray(inputs["bq"], np.float32)
    Wk = np.asarray(inputs["Wk"], np.float32)
    bk = np.asarray(inputs["bk"], np.float32)
    Wv = np.asarray(inputs["Wv"], np.float32)
    bv = np.asarray(inputs["bv"], np.float32)
    Wo = np.asarray(inputs["Wo"], np.float32)
    bo = np.asarray(inputs["bo"], np.float32)

    nc = _get_nc()

    def quarters(x):
        # [S, H] activations -> per-quarter [128, NK*NB] with contiguous rows:
        # row p of quarter n holds x.T[k*128+p, n*512:(n+1)*512] for k=0..7
        xt = np.ascontiguousarray(x.T).astype(np.float16)          # [H, S]
        xr = xt.reshape(NK, P, NI, NB).transpose(2, 1, 0, 3)       # [NI,P,NK,NB]
        return [np.ascontiguousarray(xr[n].reshape(P, NK * NB)) for n in range(NI)]

    def wrearr(wT):
        # [H, DL] -> [128, NK*DL] row-contiguous
        return np.ascontiguousarray(
            wT.reshape(NK, P, DL).transpose(1, 0, 2).reshape(P, NK * DL))

    xT = {}
    for b in range(2):
        xT[b] = (quarters(q[b]), quarters(k[b]), quarters(v[b]))

    in_maps = []
    for c in range(8):
        b, g = divmod(c, 4)
        sl = slice(g * DL, (g + 1) * DL)
        bias = np.stack(
            [bq[sl][:P], bq[sl][P:], bk[sl][:P], bk[sl][P:]], axis=1
        ).astype(np.float32)
        woTr = np.ascontiguousarray(Wo[:, sl].T).astype(np.float16)   # [DL, H]
        m = {
            "wqT": wrearr(np.ascontiguousarray(Wq[sl, :].T).astype(np.float16)),
            "wkT": wrearr(np.ascontiguousarray(Wk[sl, :].T).astype(np.float16)),
            "wvT": wrearr(np.ascontiguousarray(Wv[sl, :].T).astype(np.float16)),
            "woT": np.ascontiguousarray(
                woTr.reshape(2, P, H).transpose(1, 0, 2).reshape(P, 2 * H)),
            "bias": bias,
        }
        for n in range(NI):
            m[f"xq{n}"] = xT[b][0][n]
            m[f"xk{n}"] = xT[b][1][n]
            m[f"xv{n}"] = xT[b][2][n]
        in_maps.append(m)

    res = run_bass_kernel_spmd(
        nc, in_maps, core_ids=list(range(8)), trace=trace,
    )
    outs = [r["out"] for r in res.results]

    const = (Wo @ bv + bo).astype(np.float32)  # [1024]
    full = np.empty((2, S, H), np.float32)
    for b in range(2):
        acc = outs[4 * b].astype(np.float32).copy()
        for g in range(1, 4):
            acc += outs[4 * b + g]
        full[b] = acc.T + const
    return full, res


def kernel(**inputs):
    full, _ = run(inputs, trace=False)
    return full


# revision 86
# speedup vs baseline: 1.0218x; 1.0218x over previous
"""Multi-head attention (B=2, S=2048, H=1024, 16 heads) on 8 TRN2 NeuronCores.

Sharding: data-parallel over batch (2) x tensor-parallel over heads (16 -> 4
groups of 4 heads).  Core c = b*4 + g handles batch b, heads [4g, 4g+4).

Per-core math (fp16 storage / fp32 accumulate), with x = q|k|v of its batch:
  QP_T[d, s] = (Wq_g x^T + bq_g)   stored transposed, d on partitions
  KP_T[d, s] = (Wk_g x^T + bk_g)
  VP[s, d]   = x Wv_g^T            natural layout  (bv folded on host)
  per head h:  S_T[j, i] = KP_T_h^T-contracted scores (d contracts)
               A = exp(S_T / 8)                       (softmax w/o max-sub)
               O_T[d, i] = VP_h^T A   and  L[i] = ones^T A  (via ones col)
               O_norm = O_T * (1/L)   (1/L = exp(-ln L) on ACT, one row;
                                       partition-broadcast via DRAM DMA)
  out_T[o, i] = Wo_g^T-contracted projection of O_norm   -> [1024, 2048] f32
Host: out[b] = sum_g out_T(b,g)^T + (Wo @ bv + bo).

Schedule notes (from perfetto iteration):
- The scalar-engine exp stream (~16.8M exps/core ~= 147us) is the hard
  bottleneck; scores+exp run at high priority, everything else fills.
- Norm chains run at medium priority and drift a few slots into the next
  call; oa/ob PSUM is released early via an SBUF copy (oacp).
- Inputs are host-pre-rearranged so every DMA row is contiguous (128
  descriptors/transfer), ordered by consumption deadline; dummy exp warms
  the ACT table and dummy matmuls warm the PE clock during the DMA head.
"""

import json

import numpy as np

S = 2048
H = 1024
DL = 256          # local projection dim = 4 heads * 64
P = 128
HD = 64
NK = H // P       # 8 k-tiles over hidden dim
NI = 4            # i blocks of 512 queries
NJ = S // P       # 16 j tiles of 128 keys
NB = 512          # free-dim block

_nc_cache = {}


# --------------------------------------------------------------------------
# BIR fix: this container's walrus supports only ONE sync wait (and update)
# per TPB instruction; Tile attaches several.  Split extras onto single-wait
# EventSemaphore instructions at the serialization boundary.
# --------------------------------------------------------------------------
_wsplit_counter = [0]


def _mk_evsem(engine, debug, wait=None, update=None):
    _wsplit_counter[0] += 1
    return {
        "debug": debug,
        "engine": engine,
        "ins": [],
        "outs": [],
        "name": f"wsplit-{_wsplit_counter[0]}",
        "opcode": "EventSemaphore",
        "sync_info": {
            "on_wait": [wait] if wait else [],
            "on_update": [update] if update else [],
        },
    }


def _split_bir_waits(bir):
    for f in bir.get("functions", []):
        for blk in f.get("blocks", []):
            out = []
            for inst in blk.get("instructions", []):
                si = inst.get("sync_info")
                waits = list(si.get("on_wait") or []) if si else []
                updates = list(si.get("on_update") or []) if si else []
                eng = inst.get("engine")
                dbg = inst.get("debug", 0)
                if len(waits) > 1:
                    for w in waits[:-1]:
                        out.append(_mk_evsem(eng, dbg, wait=w))
                    si["on_wait"] = [waits[-1]]
                out.append(inst)
                if len(updates) > 1:
                    si["on_update"] = [updates[0]]
                    for u in updates[1:]:
                        out.append(_mk_evsem(eng, dbg, update=u))
            blk["instructions"] = out
    return bir


def _install_bir_fix():
    import concourse.bass as bass

    if getattr(bass.Bass, "_wsplit_installed", False):
        return
    orig = bass.Bass.to_json_bytes

    def to_json_bytes(self, *a, **k):
        bir = json.loads(orig(self, *a, **k))
        return json.dumps(_split_bir_waits(bir)).encode()

    bass.Bass.to_json_bytes = to_json_bytes
    bass.Bass._wsplit_installed = True


# --------------------------------------------------------------------------
# Kernel builder
# --------------------------------------------------------------------------

def _build_nc():
    import concourse.bass as bass
    import concourse.mybir as mybir
    import concourse.tile as tile

    f16 = mybir.dt.float16
    f32 = mybir.dt.float32
    f32r = mybir.dt.float32r
    Exp = mybir.ActivationFunctionType.Exp
    Ln = mybir.ActivationFunctionType.Ln
    VW = 4 * (HD + 1) + 63  # vp row width: 4 heads x 65 + pad to let lhsT span 128

    nc = bass.Bass("TRN2")

    # inputs arrive pre-rearranged on the host so every DMA row is contiguous
    # (128 descriptors per transfer instead of 1024 -> ~5x cheaper issue)
    xqd = [nc.dram_tensor(f"xq{n}", [P, NK * NB], f16, kind="ExternalInput")
           for n in range(NI)]
    xkd = [nc.dram_tensor(f"xk{n}", [P, NK * NB], f16, kind="ExternalInput")
           for n in range(NI)]
    xvd = [nc.dram_tensor(f"xv{n}", [P, NK * NB], f16, kind="ExternalInput")
           for n in range(NI)]
    wqT = nc.dram_tensor("wqT", [P, NK * DL], f16, kind="ExternalInput")
    wkT = nc.dram_tensor("wkT", [P, NK * DL], f16, kind="ExternalInput")
    wvT = nc.dram_tensor("wvT", [P, NK * DL], f16, kind="ExternalInput")
    woT = nc.dram_tensor("woT", [P, 2 * H], f16, kind="ExternalInput")
    bias = nc.dram_tensor("bias", [P, 4], f32, kind="ExternalInput")  # bq0 bq1 bk0 bk1
    out = nc.dram_tensor("out", [H, S], f16, kind="ExternalOutput")
    # per-call 1/l staging row for the partition-broadcast DMA
    rsc = [nc.dram_tensor(f"rsc{i}", [1, 2 * NB], f16, kind="Internal")
           for i in range(8)]

    with tile.TileContext(nc) as tc:
        with (
            tc.tile_pool(name="persist", bufs=1) as persist,
            tc.tile_pool(name="xpool", bufs=1) as xpool,
            tc.tile_pool(name="exppool", bufs=8) as exppool,
            tc.tile_pool(name="lrpool", bufs=1) as lrpool,
            tc.tile_pool(name="oacpool", bufs=2) as oacpool,
            tc.tile_pool(name="evpool", bufs=4) as evpool,
            tc.tile_pool(name="warmpool", bufs=1) as warmpool,
            tc.tile_pool(name="scps", bufs=2, space="PSUM") as scps,
            tc.tile_pool(name="oaob", bufs=1, space="PSUM") as oaob,
            tc.tile_pool(name="trans", bufs=2, space="PSUM") as trans,
        ):
            # ---- persistent tiles ----
            wq_sb = persist.tile([P, NK, DL], f16, name="wq_sb")
            wk_sb = persist.tile([P, NK, DL], f16, name="wk_sb")
            wv_sb = persist.tile([P, NK, DL], f16, name="wv_sb")
            wo_sb = persist.tile([P, 2, H], f16, name="wo_sb")
            wrm16 = persist.tile([P, NB], f16, name="wrm16")
            bias_sb = persist.tile([P, 4], f32, name="bias_sb")
            ones_f32 = persist.tile([P, P], f32, name="ones_f32")
            onesrow = persist.tile([P, P], f32r, name="onesrow")
            qpt = persist.tile([P, 2, S], f16, name="qpt")
            kpt = persist.tile([P, 2, S], f16, name="kpt")
            vp = persist.tile([P, NJ, VW], f16, name="vp")
            onorm = persist.tile([P, 2, S], f16, name="onorm")

            # ---- no-dependency setup: constants + ACT exp-table prefetch ----
            nc.gpsimd.memset(ones_f32[:], 1.0)
            nc.vector.tensor_copy(onesrow[:], ones_f32[:])
            for h in range(4):
                nc.gpsimd.memset(vp[:, :, h * (HD + 1) + HD:h * (HD + 1) + HD + 1], 1.0)
            nc.gpsimd.memset(vp[:, :, 4 * (HD + 1):], 0.0)
            # dummy exp warms the ACT table RAM (~2.7us) off the critical path
            warm = warmpool.tile([P, NB], f16, name="warm_t")
            nc.scalar.activation(warm[0:1, 0:16], ones_f32[0:1, 0:16], Exp)
            # dummy matmuls warm the PE clock (HAM K=8/8) during the DMA wait
            nc.gpsimd.memset(wrm16[:], 0.0)
            wps = trans.tile([P, NB], f32, name="tr_t")
            for i in range(12):
                nc.tensor.matmul(wps[:], wrm16[:, 0:P], wrm16[:],
                                 start=(i == 0), stop=(i == 11))
            nc.vector.tensor_copy(warm[0:1, 256:], wps[0:1, 256:])

            def warm2():
                # keep the PE busy between K proj and the xq0 arrival
                w2 = trans.tile([P, NB], f32, name="tr_t")
                for i in range(6):
                    nc.tensor.matmul(w2[:], wrm16[:, 0:P], wrm16[:],
                                     start=(i == 0), stop=(i == 5))
                nc.vector.tensor_copy(warm[0:1, 0:256], w2[0:1, 0:256])

            # ---- DMA emission, ordered by consumption deadline ----
            xk_t = {}
            xq_t = {}
            xv_t = {}

            def load_quarter(dst, src, tag, n, eng=None):
                t = xpool.tile([P, NK, NB], f16, name=f"x{tag}{n}")
                (eng or nc.sync).dma_start(
                    t[:].rearrange("p k s -> p (k s)"), src[n][:])
                dst[n] = t

            nc.sync.dma_start(bias_sb[:], bias[:])
            nc.sync.dma_start(wq_sb[:].rearrange("p k d -> p (k d)"), wqT[:])
            load_quarter(xq_t, xqd, "q", 0)
            nc.sync.dma_start(wk_sb[:].rearrange("p k d -> p (k d)"), wkT[:])
            load_quarter(xk_t, xkd, "k", 0)
            load_quarter(xk_t, xkd, "k", 1)
            nc.sync.dma_start(wv_sb[:].rearrange("p k d -> p (k d)"), wvT[:])
            load_quarter(xv_t, xvd, "v", 0)
            load_quarter(xk_t, xkd, "k", 2)
            load_quarter(xv_t, xvd, "v", 1)
            load_quarter(xk_t, xkd, "k", 3)
            load_quarter(xv_t, xvd, "v", 2)
            load_quarter(xq_t, xqd, "q", 1)
            load_quarter(xv_t, xvd, "v", 3)
            load_quarter(xq_t, xqd, "q", 2)
            load_quarter(xq_t, xqd, "q", 3)
            nc.sync.dma_start(wo_sb[:].rearrange("p k d -> p (k d)"), woT[:])

            # ---- V projection for one s-tile ----
            def v_block(s):
                n, c = divmod(s, 4)
                ps = trans.tile([P, NB], f32, name="tr_t")
                for k in range(NK):
                    nc.tensor.matmul(
                        ps[:, :DL],
                        xv_t[n][:, k, c * P:(c + 1) * P],
                        wv_sb[:, k, :],
                        start=(k == 0),
                        stop=(k == NK - 1),
                    )
                dst = vp[:, s, 0:4 * (HD + 1)].rearrange(
                    "p (h d) -> p h d", h=4)[:, :, 0:HD]
                nc.vector.tensor_copy(dst, ps[:, :DL].rearrange("p (h d) -> p h d", h=4))

            # ---- K/Q projection, one (m, n) block ----
            def proj_qk_n(wsb, xt, dst, bcol, m, n, split_tsa=False):
                ps = trans.tile([P, NB], f32, name="tr_t")
                for k in range(NK):
                    nc.tensor.matmul(
                        ps[:],
                        wsb[:, k, m * P:(m + 1) * P],
                        xt[n][:, k, :],
                        start=(k == 0),
                        stop=(k == NK - 1),
                    )
                if split_tsa:
                    # 128-col chunks: the first scores matmul needs only the
                    # first jb slice, so it fires ~0.5us after the proj
                    for c in range(4):
                        nc.vector.tensor_scalar_add(
                            dst[:, m, n * NB + c * P:n * NB + (c + 1) * P],
                            ps[:, c * P:(c + 1) * P],
                            bias_sb[:, bcol + m:bcol + m + 1],
                        )
                else:
                    nc.vector.tensor_scalar_add(
                        dst[:, m, n * NB:(n + 1) * NB], ps[:],
                        bias_sb[:, bcol + m:bcol + m + 1],
                    )

            # ---- output projection, one mo chunk of one n block ----
            def wo_chunk(n, mo, ceng=None):
                ps = trans.tile([P, NB], f32, name="tr_t")
                for k2 in range(2):
                    nc.tensor.matmul(
                        ps[:],
                        wo_sb[:, k2, mo * P:(mo + 1) * P],
                        onorm[:, k2, n * NB:(n + 1) * NB],
                        start=(k2 == 0),
                        stop=(k2 == 1),
                    )
                ot = evpool.tile([P, NB], f16, name="ot_t")
                if ceng is None:
                    nc.vector.tensor_copy(ot[:], ps[:])
                else:
                    nc.scalar.copy(ot[:], ps[:])
                nc.sync.dma_start(out[mo * P:(mo + 1) * P, n * NB:(n + 1) * NB], ot[:])

            HP = 3000   # priority offset: scores/exp chain preempts fill work
            HP2 = 2930  # norm chain: a few slots into the next call's stream

            def attention_ib(p, ib, fills=None, norm_hp=None, last=False):
                isl = slice(ib * NB, (ib + 1) * NB)
                oa = oaob.tile([P, NB], f32, name="oa_t")
                ob = oaob.tile([P, NB], f32, name="ob_t")
                es = []

                def emit_av(jb):
                    e = es[jb]
                    offa = 2 * p * (HD + 1)
                    offb = (2 * p + 1) * (HD + 1)
                    nc.tensor.matmul(
                        oa[:, :], vp[:, jb, offa:offa + P], e[:, 0:NB],
                        start=(jb == 0), stop=(jb == NJ - 1),
                    )
                    nc.tensor.matmul(
                        ob[:, :], vp[:, jb, offb:offb + P], e[:, NB:2 * NB],
                        start=(jb == 0), stop=(jb == NJ - 1),
                    )

                # one-stage software pipeline: QK(jb) ahead of AV(jb-1)
                for jb in range(NJ):
                    jsl = slice(jb * P, (jb + 1) * P)
                    sc = scps.tile([P, 2 * NB], f32, name="sc_t")
                    with tc.high_priority(offset=HP):
                        nc.tensor.matmul(
                            sc[:, 0:NB], kpt[0:HD, p, jsl], qpt[0:HD, p, isl],
                            start=True, stop=True,
                        )
                        nc.tensor.matmul(
                            sc[:, NB:2 * NB], kpt[HD:P, p, jsl], qpt[HD:P, p, isl],
                            start=True, stop=True,
                        )
                        e = exppool.tile([P, 2 * NB], f16, name="e_t")
                        nc.scalar.activation(e[:], sc[:], Exp, scale=0.125)
                    es.append(e)
                    if fills is not None and jb in fills:
                        for th in fills[jb]:
                            th()
                    if jb >= 1:
                        emit_av(jb - 1)
                emit_av(NJ - 1)

                if last:
                    # tail call: PSUM is free and latency is king — broadcast
                    # l with K=1 matmuls, 1/l on the broadcast tile, multiply.
                    with tc.high_priority(offset=HP):
                        lsb = lrpool.tile([P, 2 * NB], f32r, name="lsb_t")
                        with nc.allow_low_precision(reason="denom via f32r"):
                            nc.vector.tensor_copy(lsb[HD:HD + 1, 0:NB],
                                                  oa[HD:HD + 1, :])
                            nc.vector.tensor_copy(lsb[HD:HD + 1, NB:2 * NB],
                                                  ob[HD:HD + 1, :])
                        rb = scps.tile([P, 2 * NB], f32, name="sc_t")
                        nc.tensor.matmul(
                            rb[:, 0:NB], onesrow[HD:HD + 1, :],
                            lsb[HD:HD + 1, 0:NB], start=True, stop=True)
                        nc.tensor.matmul(
                            rb[:, NB:2 * NB], onesrow[HD:HD + 1, :],
                            lsb[HD:HD + 1, NB:2 * NB], start=True, stop=True)
                        lnb = lrpool.tile([P, 2 * NB], f32, name="lnb_t")
                        nc.scalar.activation(lnb[:], rb[:], Ln)
                        rbs = lrpool.tile([P, 2 * NB], f32, name="rbs_t")
                        nc.scalar.activation(rbs[:], lnb[:], Exp, scale=-1.0)
                        tmpb = evpool.tile([P, NB], f16, name="tmpb_t")
                        nc.vector.tensor_mul(tmpb[0:HD, :], ob[0:HD, :],
                                             rbs[0:HD, NB:2 * NB])
                        nc.sync.dma_start(onorm[HD:P, p, isl], tmpb[0:HD, :])
                        nc.vector.tensor_mul(onorm[0:HD, p, isl], oa[0:HD, :],
                                             rbs[0:HD, 0:NB])
                    return

                # normalization: l sits at row HD of oa/ob.  Copy O|l out of
                # PSUM immediately (releases oa/ob for the next call), then
                # 1/l = exp(-ln l) on the single l row (ACT, natural_log_exp
                # set), broadcast r across partitions via a DRAM-staged DMA,
                # multiply on the DVE.  Medium priority: next call's exp
                # stream preempts, the chain fills ACT/PE bubbles.
                with tc.high_priority(offset=HP):
                    oacp = oacpool.tile([P, 2 * NB], f32, name="oacp_t")
                    nc.vector.tensor_copy(oacp[0:HD + 1, 0:NB], oa[0:HD + 1, :])
                    nc.vector.tensor_copy(oacp[0:HD + 1, NB:2 * NB], ob[0:HD + 1, :])
                with tc.high_priority(offset=HP2 if norm_hp is None else norm_hp):
                    lrow = lrpool.tile([P, 2 * NB], f32, name="lrow_t")
                    nc.scalar.activation(lrow[HD:HD + 1, :], oacp[HD:HD + 1, :], Ln)
                    rrow = lrpool.tile([P, 2 * NB], f32, name="rrow_t")
                    nc.scalar.activation(rrow[HD:HD + 1, :], lrow[HD:HD + 1, :], Exp,
                                         scale=-1.0)
                    # partition-broadcast 1/l: SWDGE cast-DMA to a DRAM row,
                    # then a zero-stride read fans it across 128 partitions
                    # (SBUF sources cannot have partition step 0; DRAM can)
                    rdram = rsc[4 * p + ib]
                    nc.gpsimd.dma_start(rdram[:], rrow[HD:HD + 1, :])
                    rbsb = lrpool.tile([P, 2 * NB], f16, name="rbsb_t")
                    nc.gpsimd.dma_start(
                        rbsb[:], rdram[:].to_broadcast([P, 2 * NB]))
                    tmpb = evpool.tile([P, NB], f16, name="tmpb_t")
                    nc.vector.tensor_mul(
                        tmpb[0:HD, :], oacp[0:HD, NB:2 * NB], rbsb[0:HD, NB:2 * NB])
                    nc.sync.dma_start(onorm[HD:P, p, isl], tmpb[0:HD, :])
                    nc.vector.tensor_mul(
                        onorm[0:HD, p, isl], oacp[0:HD, 0:NB], rbsb[0:HD, 0:NB])

            # ---- schedule ----
            def P_(wsb, xd, dst, bcol, m, n):
                return lambda: proj_qk_n(wsb, xd, dst, bcol, m, n)

            K0 = lambda n: P_(wk_sb, xk_t, kpt, 2, 0, n)
            Q0 = lambda n: P_(wq_sb, xq_t, qpt, 0, 0, n)
            K1 = lambda n: P_(wk_sb, xk_t, kpt, 2, 1, n)
            Q1 = lambda n: P_(wq_sb, xq_t, qpt, 0, 1, n)
            V_ = lambda s: (lambda: v_block(s))
            WO = lambda n, mo: (lambda: wo_chunk(n, mo))

            def wo_fills(n):
                return {2 * mo + 1: [WO(n, mo)] for mo in range(8)}

            # prologue: only what gates the first exp; V rides as call-0
            # fills.  Q first (its x lands first), K second with a chunked
            # bias-add so sc(0) fires off the first 128 columns of kpt.
            Q0(0)()
            warm2()
            proj_qk_n(wk_sb, xk_t, kpt, 2, 0, 0, split_tsa=True)

            attention_ib(0, 0, fills={
                0: [K0(1)],
                1: [V_(0)], 2: [V_(1)], 3: [V_(2)],
                4: [K0(2), V_(3)],
                5: [V_(4), V_(5)],
                6: [V_(6)], 7: [V_(7)],
                8: [K0(3), V_(8)],
                9: [V_(9)], 10: [V_(10)], 11: [V_(11)],
                12: [Q0(1), V_(12)],
                13: [V_(13)], 14: [V_(14)], 15: [V_(15)],
            })
            attention_ib(0, 1, fills={2: [Q0(2)], 8: [K1(0)], 12: [K1(1)]})
            attention_ib(0, 2, fills={2: [Q0(3)], 8: [K1(2)], 12: [K1(3)]})
            attention_ib(0, 3, fills={2: [Q1(0)], 8: [Q1(1)], 12: [Q1(2)]})
            attention_ib(1, 0, fills={2: [Q1(3)]})
            attention_ib(1, 1, fills=wo_fills(0))
            attention_ib(1, 2, fills=wo_fills(1))
            attention_ib(1, 3, fills={jb + 7: [WO(2, jb)] for jb in range(8)},
                         last=True)
            # tail PE warmers from the now-free scps pool: keep HAM at 8/8
            # through the last norm chain so wo(3) runs at full clock
            wps3 = scps.tile([P, 2 * NB], f32, name="sc_t")
            for i in range(14):
                nc.tensor.matmul(wps3[:, 0:NB], wrm16[:, 0:P], wrm16[:],
                                 start=(i == 0), stop=(i == 13))
            nc.vector.tensor_copy(warm[0:1, 0:128], wps3[0:1, 0:128])
            for mo in range(8):
                wo_chunk(3, mo)

    return nc


def _get_nc():
    if "nc" not in _nc_cache:
        _install_bir_fix()
        _nc_cache["nc"] = _build_nc()
    return _nc_cache["nc"]


# --------------------------------------------------------------------------
# Host wrapper
# --------------------------------------------------------------------------
def run(inputs, trace=False):
    from concourse.bass_utils import run_bass_kernel_spmd

    q = np.asarray(inputs["q"], np.float32)
    k = np.asarray(inputs["k"], np.float32)
    v = np.asarray(inputs["v"], np.float32)
    Wq = np.asarray(inputs["Wq"], np.float32)
    bq = np.asarray(inputs["bq"], np.float32)
    Wk = np.asarray(inputs["Wk"], np.float32)
    bk = np.asarray(inputs["bk"], np.float32)
    Wv = np.asarray(inputs["Wv"], np.float32)
    bv = np.asarray(inputs["bv"], np.float32)
    Wo = np.asarray(inputs["Wo"], np.float32)
    bo = np.asarray(inputs["bo"], np.float32)

    nc = _get_nc()

    def quarters(x):
        # [S, H] activations -> per-quarter [128, NK*NB] with contiguous rows:
        # row p of quarter n holds x.T[k*128+p, n*512:(n+1)*512] for k=0..7
        xt = np.ascontiguousarray(x.T).astype(np.float16)          # [H, S]
        xr = xt.reshape(NK, P, NI, NB).transpose(2, 1, 0, 3)       # [NI,P,NK,NB]
        return [np.ascontiguousarray(xr[n].reshape(P, NK * NB)) for n in range(NI)]

    def wrearr(wT):
        # [H, DL] -> [128, NK*DL] row-contiguous
        return np.ascontiguousarray(
            wT.reshape(NK, P, DL).transpose(1, 0, 2).reshape(P, NK * DL))

    xT = {}
    for b in range(2):
        xT[b] = (quarters(q[b]), quarters(k[b]), quarters(v[b]))

    in_maps = []
    for c in range(8):
        b, g = divmod(c, 4)
        sl = slice(g * DL, (g + 1) * DL)
        bias = np.stack(
            [bq[sl][:P], bq[sl][P:], bk[sl][:P], bk[sl][P:]], axis=1
        ).astype(np.float32)
        woTr = np.ascontiguousarray(Wo[:, sl].T).astype(np.float16)   # [DL, H]
        m = {
            "wqT": wrearr(np.ascontiguousarray(Wq[sl, :].T).astype(np.float16)),
            "wkT": wrearr(np.ascontiguousarray(Wk[sl, :].T).astype(np.float16)),
            "wvT": wrearr(np.ascontiguousarray(Wv[sl, :].T).astype(np.float16)),
            "woT": np.ascontiguousarray(
                woTr.reshape(2, P, H).transpose(1, 0, 2).reshape(P, 2 * H)),
            "bias": bias,
        }
        for n in range(NI):
            m[f"xq{n}"] = xT[b][0][n]
            m[f"xk{n}"] = xT[b][1][n]
            m[f"xv{n}"] = xT[b][2][n]
        in_maps.append(m)

    res = run_bass_kernel_spmd(
        nc, in_maps, core_ids=list(range(8)), trace=trace,
    )
    outs = [r["out"] for r in res.results]

    const = (Wo @ bv + bo).astype(np.float32)  # [1024]
    full = np.empty((2, S, H), np.float32)
    for b in range(2):
        acc = outs[4 * b].astype(np.float32).copy()
        for g in range(1, 4):
            acc += outs[4 * b + g]
        full[b] = acc.T + const
    return full, res


def kernel(**inputs):
    full, _ = run(inputs, trace=False)
    return full
